# revision 34
# baseline (speedup 1.0000x reference)
"""Trainium2 Bass/Tile kernel for the GatedNode2Edge op.

Computes, for emb (B,C,N), th12_* (E,C), th5_* (E,):
    t_k  = th12_k @ emb[b]                      (E,N)
    m_k  = max(t_k[:,i], t_k[:,j]) pairwise     (E,N,N)
    adj  = relu(2*m_1 + th5_1*I)
    gate = sigmoid(relu(2*m_2 + th5_2*I))
    out  = adj * gate                           (B,E,N,N)

Sharding: the 64 (b,e) channels are split 8-per-core across 8 NeuronCores.

Math restructuring (off-diagonal), with v = 2*relu(t1), g = sigmoid(2*relu(t2)):
    out[i,j] = max(v_i, v_j) * max(g_i, g_j)
one fused custom-DVE op per [128, N] output tile:
    out = maxx(Src0, C0) * maxx(Src1, C1)
Src0/Src1 = v/g replicated across partitions (built by an indicator-matmul
on the PE: lhsT[k,m] = (k==ch) selects channel ch's row, bf16 at 1 cyc/row),
C0/C1 = per-partition column slices. The custom op carries a hand-authored
2X_1PORT uop program (two parallel max/max/mul chains over the packed bf16
pair via SRC_*_HI, written through WR0_LO/WR0_HI), doubling DVE throughput
to ~2 elem/lane/cycle — the single biggest win in this kernel.

The true diagonal d1*d2 (d1 = relu(2t1+th5_1), d2 = sigmoid(relu(2t2+th5_2)))
is patched per channel with ONE strided copy_predicated over the [128, 8, 128]
diagonal view of the channel's [128, 8*N] mega-tile (hand-built AP with free
pattern [[N+128, 8], [1, 128]]), amortizing per-op overhead 8x.

Everything on-device is bf16 (harness tolerance 2e-2 >> bf16 rounding); the
host converts back to f32. This halves HBM write traffic (the DMA roofline)
and halves matmul LDWEIGHTS time. Stores are deferred one channel so the
in-order sync/scalar sequencers never stall Vector on a not-yet-patched
tile; the last channel stores per-tile to overlap the drain.
"""

import sys
import types

import ml_dtypes
import numpy as np

B, C, N, E = 2, 64, 1024, 32
NCORES = 8
EPC = B * E // NCORES  # 8 channels per core
P = 128
NB = N // P  # 8 row blocks
H = N // 2  # matmul moving free-dim limit is 512

# Try to engage the DVE 2X_1PORT perf mode for the custom op (bf16 packed
# operands). The same uop program is written into the perf-mode table
# slots; correctness is checked end-to-end by the harness.
USE_2X = True
_CACHE = {}


def _ensure_hook_shim():
    """Make trace=True safe even when antenv.axon_hooks is absent."""
    try:
        import antenv.axon_hooks  # noqa: F401
    except ImportError:
        mod = types.ModuleType("antenv.axon_hooks")
        mod.get_axon_ntff_profile_hook = lambda: None
        mod.set_axon_ntff_profile_hook = lambda h: None
        sys.modules["antenv.axon_hooks"] = mod


def _build_2x_uop(base):
    """2X_1PORT program for out = max(in0,s0)*max(in1,s1).

    In 2X mode each port read delivers a packed pair of bf16 elements; the
    low element enters via SRC_0/SRC_1, the high one via SRC_0_HI/SRC_1_HI.
    Two copies of the max/max/mul chain are placed across the 8 ALU blocks;
    results are written packed via WR0_LO / WR0_HI. Per-partition scalar
    consts are shared by both chains (both elements are in the same row).
    """
    from concourse.dve_uop import (
        AluInp, AluOp, DelayInp, ENABLE, InpSel, OutPath, OutSel, UopConfig,
    )

    u = UopConfig()
    # lane j>=1 feeds delay chain j-1 at block 0
    u.enable_input(InpSel.SRC_0, 1)      # chain 0
    u.enable_input(InpSel.CONST_0, 2)    # chain 1
    u.enable_input(InpSel.SRC_1, 3)      # chain 2
    u.enable_input(InpSel.CONST_1, 4)    # chain 3
    u.enable_input(InpSel.SRC_0_HI, 5)   # chain 4
    u.enable_input(InpSel.SRC_1_HI, 6)   # chain 5
    u.require_inp0 = ENABLE
    u.require_inp1 = ENABLE
    u.trigger = base.trigger
    u.next_uop = (0, 0, 0)
    u.repeat_count = base.repeat_count

    dp = u.datapath_config
    # low chain
    dp[0].enable_alu(AluOp.MAX, AluInp.PREV_DELAY_0, AluInp.PREV_DELAY_1)
    dp[0].pass_through_delay(1, 2, 3, 4, 5)
    dp[1].enable_alu(AluOp.MAX, AluInp.PREV_DELAY_2, AluInp.PREV_DELAY_3)
    dp[1].enable_delay_from_src(DelayInp.PREV_ALU_OUT, 0)  # max_v_lo
    dp[1].pass_through_delay(1, 3, 4, 5)
    dp[2].enable_alu(AluOp.MULTIPLY, AluInp.PREV_DELAY_0, AluInp.PREV_ALU_OUT)
    dp[2].pass_through_delay(1, 3, 4, 5)
    # high chain (consts still on chains 1 and 3)
    dp[3].enable_alu(AluOp.MAX, AluInp.PREV_DELAY_4, AluInp.PREV_DELAY_1)
    dp[3].enable_delay_from_src(DelayInp.PREV_ALU_OUT, 0)  # out_lo
    dp[3].pass_through_delay(3, 5)
    dp[4].enable_alu(AluOp.MAX, AluInp.PREV_DELAY_5, AluInp.PREV_DELAY_3)
    dp[4].enable_delay_from_src(DelayInp.PREV_ALU_OUT, 1)  # max_v_hi
    dp[4].pass_through_delay(0)
    dp[5].enable_alu(AluOp.MULTIPLY, AluInp.PREV_DELAY_1, AluInp.PREV_ALU_OUT)
    dp[5].pass_through_delay(0)
    dp[6].pass_through_alu()
    dp[6].pass_through_delay(0)
    dp[7].pass_through_alu()
    dp[7].pass_through_delay(0)

    u.out[OutPath.WR0_LO] = OutSel.DELAY_0
    u.out_enable[OutPath.WR0_LO] = ENABLE
    u.out[OutPath.WR0_HI] = OutSel.ALU_OUT
    u.out_enable[OutPath.WR0_HI] = ENABLE
    return u


def _register_gated_maxmul():
    """Register the fused out = max(in0,s0)*max(in1,s1) custom DVE op."""
    import concourse.dve_ops as dve_ops
    from concourse.dve_ops import DveOp, OPS, has_src1, _COMPILE_CACHE
    from concourse.dve_spec import C0, C1, Spec, Src0, Src1, lower, maxx
    from concourse.dve_uop import DveOpSpec

    for op in OPS:
        if op.name == "GATED_MAXMUL_ANT":
            return op

    spec = Spec(
        body=maxx(Src0, C0) * maxx(Src1, C1),
        reference=lambda in0, in1, s0, s1, imm2: np.maximum(in0, s0)
        * np.maximum(in1, s1),
    )
    op = DveOp("GATED_MAXMUL_ANT", spec, subdim=False, uops_sha={})
    OPS.append(op)
    # Rebuild the registry views that were snapshotted at import time.
    dve_ops.CUSTOM_DVE_SPECS[op.name] = op.spec
    opcode = dve_ops._CUSTOM_DVE_ROW_BASE + len(OPS) - 1
    assert opcode < 0x20
    dve_ops._SUB_OPCODE_FOR_NAME[op.name] = opcode
    # Pre-seed the compile cache with a spec that (optionally) carries the
    # perf-mode uop programs; compile() then returns it without the sha check.
    for ver in ("v3", "v4"):
        uops = lower(spec, ver=ver)
        kw = {}
        if USE_2X:
            kw = dict(uops_2x=[_build_2x_uop(uops[0])])
        s = DveOpSpec(
            name=op.name, opcode=opcode, uops=uops,
            rd1_en=has_src1(spec), **kw,
        )
        op.uops_sha[ver] = s.sha(ver)
        _COMPILE_CACHE[(op.name, ver)] = s
    return op


def _build_program():
    import concourse.bacc as bacc
    import concourse.mybir as mybir
    import concourse.tile as tile
    from concourse.ap import AP

    f32 = mybir.dt.float32
    bf16 = mybir.dt.bfloat16
    AF = mybir.ActivationFunctionType
    ALU = mybir.AluOpType

    gated_op = _register_gated_maxmul()

    nc = bacc.Bacc("TRN2", target_bir_lowering=False, debug=False, num_devices=NCORES)

    emb = nc.declare_dram_parameter("emb", [C, N], bf16, isOutput=False)
    w = nc.declare_dram_parameter("w", [C, 40], bf16, isOutput=False)
    sel = nc.declare_dram_parameter("sel", [40, N], bf16, isOutput=False)
    th5bc = nc.declare_dram_parameter("th5bc", [P, NB * 2 * EPC], f32, isOutput=False)
    eye8 = nc.declare_dram_parameter("eye8", [P, NB * P], bf16, isOutput=False)
    out = nc.declare_dram_parameter("out", [EPC, N, N], bf16, isOutput=True)

    def custom(out_ap, in0, in1, s0, s1):
        bi = nc.vector._custom_dve(gated_op, out=out_ap, in0=in0, in1=in1, s0=s0, s1=s1)
        if USE_2X:
            bi.ins.perf_max = 1  # engine may escalate to 2X_1PORT
        return bi

    with tile.TileContext(nc, pool_alloc_mode="queue") as tc:
        with (
            tc.tile_pool(name="const", bufs=1) as cpool,
            tc.tile_pool(name="rows", bufs=1) as rpool,
        ):
            sb_emb = cpool.tile([C, N], bf16)
            Q = N // 4
            for q in range(4):
                nc.sync.dma_start(out=sb_emb[:, q * Q:(q + 1) * Q],
                                  in_=emb[:, q * Q:(q + 1) * Q])
            sb_w = cpool.tile([C, 40], bf16)
            nc.sync.dma_start(out=sb_w[:], in_=w[:])
            sb_sel = cpool.tile([40, N], bf16)
            nc.sync.dma_start(out=sb_sel[:], in_=sel[:])
            sb_th5bc = cpool.tile([P, NB, 2 * EPC], f32)
            nc.sync.dma_start(out=sb_th5bc[:], in_=th5bc[:])
            sb_eye8 = cpool.tile([P, NB, P], bf16)
            nc.sync.dma_start(out=sb_eye8[:], in_=eye8[:])
            # Warm the ACT sigmoid table during the input loads.
            sb_warm = cpool.tile([1, EPC], f32)
            nc.vector.memset(sb_warm[:], 0.0)
            nc.scalar.activation(sb_warm[:], sb_warm[:], AF.Sigmoid)

            # Row-space: vg rows (v on partitions 0-7, g on 32-39), bf16.
            # g sits at partition 32 because engine APs must start on a
            # quad (32-partition) boundary.
            sb_vg = rpool.tile([40, N], bf16)
            # Column-space: vgcol[:, r, 0:8] = v at node r*128+p, [:, r, 8:16] = g.
            sb_vgc = rpool.tile([P, NB, 2 * EPC], f32)
            sb_pdb = rpool.tile([P, NB, EPC], bf16)  # true diag d1*d2
            sb_u = rpool.tile([P, NB, 2 * EPC], f32)
            sb_d = rpool.tile([P, NB, 2 * EPC], f32)

            with (
                tc.tile_pool(name="psum", bufs=2, space="PSUM") as pp,
                tc.tile_pool(name="ph1ps", bufs=1, space="PSUM") as pt,
                tc.tile_pool(name="colps", bufs=1, space="PSUM") as cp,
                tc.tile_pool(name="jrepsb", bufs=2) as jsb,
                tc.tile_pool(name="work", bufs=4) as wp,
            ):
                # Rows: t = w.T @ emb -> [40, N] (t1 on 0-7, t2 on 32-39).
                ps_t = pt.tile([40, N], f32, tag="ps_t")
                for h in range(2):
                    hs = slice(h * H, (h + 1) * H)
                    nc.tensor.matmul(
                        ps_t[:, hs], lhsT=sb_w[:], rhs=sb_emb[:, hs],
                        start=True, stop=True,
                    )
                    nc.scalar.activation(
                        sb_vg[0:EPC, hs], ps_t[0:EPC, hs], AF.Relu, scale=2.0,
                    )
                    nc.scalar.activation(
                        sb_vg[32:40, hs], ps_t[32:40, hs], AF.Relu, scale=2.0,
                    )
                    nc.scalar.activation(
                        sb_vg[32:40, hs], sb_vg[32:40, hs], AF.Sigmoid,
                    )

                # Channel 0's replication goes right behind phase 1 in the
                # PE stream so its vj/gj are ready with the first columns.
                sb_vj = jsb.tile([P, N], bf16, tag="sb_vj")
                sb_gj = jsb.tile([P, N], bf16, tag="sb_gj")
                for h in range(2):
                    hs = slice(h * H, (h + 1) * H)
                    ps_vh = pp.tile([P, H], f32, tag="ps_vh")
                    nc.tensor.matmul(
                        ps_vh[:], lhsT=sb_sel[0:EPC, 0:P],
                        rhs=sb_vg[0:EPC, hs], start=True, stop=True,
                    )
                    nc.scalar.copy(sb_vj[:, hs], ps_vh[:])
                    ps_gh = pp.tile([P, H], f32, tag="ps_gh")
                    nc.tensor.matmul(
                        ps_gh[:], lhsT=sb_sel[32:40, 0:P],
                        rhs=sb_vg[32:40, hs], start=True, stop=True,
                    )
                    nc.scalar.copy(sb_gj[:, hs], ps_gh[:])
                repl0 = (sb_vj, sb_gj)

                # Columns: tcol[p, r, k] = t_k[r*128+p] via emb-block matmuls.
                ps_c = cp.tile([P, NB, 40], f32, tag="ps_c")
                for r in range(NB):
                    nc.tensor.matmul(
                        ps_c[:, r, :], lhsT=sb_emb[:, r * P:(r + 1) * P],
                        rhs=sb_w[:], start=True, stop=True,
                    )
                # vgc = relu(2*tcol) (+ sigmoid on g half); u = 2*tcol + th5.
                # In r-halves so the first GATED isn't gated on column r7.
                for a, b in ((0, NB // 2), (NB // 2, NB)):
                    nc.vector.tensor_scalar(
                        sb_vgc[:, a:b, 0:EPC], ps_c[:, a:b, 0:EPC], 2.0, 0.0,
                        op0=ALU.mult, op1=ALU.max,
                    )
                    nc.vector.tensor_scalar(
                        sb_vgc[:, a:b, EPC:], ps_c[:, a:b, 32:40], 2.0, 0.0,
                        op0=ALU.mult, op1=ALU.max,
                    )
                    nc.scalar.activation(
                        sb_vgc[:, a:b, EPC:], sb_vgc[:, a:b, EPC:], AF.Sigmoid,
                    )
                    nc.vector.scalar_tensor_tensor(
                        sb_u[:, a:b, 0:EPC], ps_c[:, a:b, 0:EPC], 2.0,
                        sb_th5bc[:, a:b, 0:EPC], op0=ALU.mult, op1=ALU.add,
                    )
                    nc.vector.scalar_tensor_tensor(
                        sb_u[:, a:b, EPC:], ps_c[:, a:b, 32:40], 2.0,
                        sb_th5bc[:, a:b, EPC:], op0=ALU.mult, op1=ALU.add,
                    )

                pending = None  # (ch, mega-tile) stores deferred one iter
                for ch in range(EPC):
                    if ch == 0:
                        sb_vj, sb_gj = repl0
                    else:
                        # Replicate channel ch's v/g rows across partitions
                        # (indicator-matmul, K=8, bf16, 1 cyc/row).
                        sb_vj = jsb.tile([P, N], bf16, tag="sb_vj")
                        sb_gj = jsb.tile([P, N], bf16, tag="sb_gj")
                        for h in range(2):
                            hs = slice(h * H, (h + 1) * H)
                            ps_vh = pp.tile([P, H], f32, tag="ps_vh")
                            nc.tensor.matmul(
                                ps_vh[:],
                                lhsT=sb_sel[0:EPC, ch * P:(ch + 1) * P],
                                rhs=sb_vg[0:EPC, hs],
                                start=True, stop=True,
                            )
                            nc.scalar.copy(sb_vj[:, hs], ps_vh[:])
                            ps_gh = pp.tile([P, H], f32, tag="ps_gh")
                            nc.tensor.matmul(
                                ps_gh[:],
                                lhsT=sb_sel[32:40, ch * P:(ch + 1) * P],
                                rhs=sb_vg[32:40, hs],
                                start=True, stop=True,
                            )
                            nc.scalar.copy(sb_gj[:, hs], ps_gh[:])

                    # Flush the previous channel's stores now: its diag patch
                    # is already done, so the sequencers won't stall on it,
                    # and the vj/gj copies above stay ahead of Vector.
                    if pending is not None:
                        pch, po = pending
                        for r in range(NB):
                            eng = nc.sync if r % 2 == 0 else nc.scalar
                            eng.dma_start(
                                out=out[pch, r * P:(r + 1) * P, :],
                                in_=po[:, r, :],
                            )
                        pending = None

                    # One mega-tile per channel; DVE fills the 8 row blocks.
                    last = ch == EPC - 1
                    o = wp.tile([P, NB, N], bf16, tag="o")
                    for r in range(NB):
                        custom(
                            o[:, r, :], sb_vj[:], sb_gj[:],
                            sb_vgc[:, r, ch:ch + 1],
                            sb_vgc[:, r, EPC + ch:EPC + ch + 1],
                        )
                        if last:
                            # Per-tile patch + store so the drain overlaps
                            # the remaining compute instead of trailing it.
                            nc.vector.copy_predicated(
                                o[:, r, r * P:(r + 1) * P],
                                sb_eye8[:, r, :].bitcast(mybir.dt.int16),
                                sb_pdb[:, r, ch:ch + 1].broadcast_to([P, P]),
                            )
                            nc.sync.dma_start(
                                out=out[ch, r * P:r * P + 64, :],
                                in_=o[0:64, r, :],
                            )
                            nc.scalar.dma_start(
                                out=out[ch, r * P + 64:(r + 1) * P, :],
                                in_=o[64:128, r, :],
                            )
                    if ch == 0:
                        # Deferred diag-value chain: relu on Vector, only the
                        # sigmoid needs Scalar, so neither engine stalls.
                        nc.vector.tensor_scalar_max(sb_d[:], sb_u[:], 0.0)
                        nc.scalar.activation(sb_d[:, :, EPC:], sb_d[:, :, EPC:],
                                             AF.Sigmoid)
                        nc.vector.tensor_mul(sb_pdb[:], sb_d[:, :, :EPC],
                                             sb_d[:, :, EPC:])

                    if not last:
                        # Batched diagonal patch: one strided op per channel.
                        # Diag of row block r lives at free offset r*N + r*P,
                        # i.e. the [[N+P, NB], [1, P]] strided view.
                        full = o[:]
                        pairs = [list(p) for p in full.ap]
                        dview = AP(full.tensor, full.offset,
                                   [pairs[0], [N + P, NB], [1, P]])
                        nc.vector.copy_predicated(
                            dview, sb_eye8[:].bitcast(mybir.dt.int16),
                            sb_pdb[:, :, ch:ch + 1].broadcast_to([P, NB, P]),
                        )
                        pending = (ch, o)

                # Flush any channel whose stores are still deferred.
                if pending is not None:
                    pch, po = pending
                    for r in range(NB):
                        eng = nc.sync if r % 2 == 0 else nc.scalar
                        eng.dma_start(out=out[pch, r * P:(r + 1) * P, :],
                                      in_=po[:, r, :])

    nc.compile()
    return nc


def _get_program():
    if "nc" not in _CACHE:
        _CACHE["nc"] = _build_program()
    return _CACHE["nc"]


def kernel(**inputs):
    _ensure_hook_shim()
    from concourse.bass_utils import run_bass_kernel_spmd

    bf = ml_dtypes.bfloat16
    emb = np.ascontiguousarray(np.asarray(inputs["emb"], dtype=np.float32)).astype(bf)
    th12_1 = np.asarray(inputs["th12_1"], dtype=np.float32)
    th12_2 = np.asarray(inputs["th12_2"], dtype=np.float32)
    th5_1 = np.asarray(inputs["th5_1"], dtype=np.float32)
    th5_2 = np.asarray(inputs["th5_2"], dtype=np.float32)
    eye8 = np.tile(np.eye(P, dtype=np.float32), (1, NB)).astype(bf)

    # sel[k, ch*128+m] = (k==ch) for k<8 and (k-32==ch) for 32<=k<40
    sel = np.zeros((40, N), dtype=bf)
    for ch in range(EPC):
        sel[ch, ch * P:(ch + 1) * P] = 1
        sel[32 + ch, ch * P:(ch + 1) * P] = 1

    in_maps = []
    for k in range(NCORES):
        b = k // (NCORES // B)
        e0 = (k % (NCORES // B)) * EPC
        w = np.zeros((C, 40), dtype=bf)
        w[:, 0:EPC] = th12_1[e0:e0 + EPC].T.astype(bf)
        w[:, 32:40] = th12_2[e0:e0 + EPC].T.astype(bf)
        th5cat = np.concatenate([th5_1[e0:e0 + EPC], th5_2[e0:e0 + EPC]])  # [16]
        th5bc = np.tile(th5cat[None, :], (P, NB)).astype(np.float32)  # [128, 128]
        in_maps.append(
            {
                "emb": np.ascontiguousarray(emb[b]),
                "w": np.ascontiguousarray(w),
                "sel": sel,
                "th5bc": th5bc,
                "eye8": eye8,
            }
        )

    nc = _get_program()
    res = run_bass_kernel_spmd(nc, in_maps, core_ids=list(range(NCORES)))
    _CACHE["last_result"] = res

    out = np.empty((B, E, N, N), dtype=np.float32)
    for k in range(NCORES):
        b = k // (NCORES // B)
        e0 = (k % (NCORES // B)) * EPC
        out[b, e0:e0 + EPC] = np.asarray(res.results[k]["out"], dtype=np.float32)
    return out


# revision 35
# speedup vs baseline: 1.0171x; 1.0171x over previous
"""Trainium2 Bass/Tile kernel for the GatedNode2Edge op.

Computes, for emb (B,C,N), th12_* (E,C), th5_* (E,):
    t_k  = th12_k @ emb[b]                      (E,N)
    m_k  = max(t_k[:,i], t_k[:,j]) pairwise     (E,N,N)
    adj  = relu(2*m_1 + th5_1*I)
    gate = sigmoid(relu(2*m_2 + th5_2*I))
    out  = adj * gate                           (B,E,N,N)

Sharding: the 64 (b,e) channels are split 8-per-core across 8 NeuronCores.

Math restructuring (off-diagonal), with v = 2*relu(t1), g = sigmoid(2*relu(t2)):
    out[i,j] = max(v_i, v_j) * max(g_i, g_j)
one fused custom-DVE op per [128, N] output tile:
    out = maxx(Src0, C0) * maxx(Src1, C1)
Src0/Src1 = v/g replicated across partitions (built by an indicator-matmul
on the PE: lhsT[k,m] = (k==ch) selects channel ch's row, bf16 at 1 cyc/row),
C0/C1 = per-partition column slices. The custom op carries a hand-authored
2X_1PORT uop program (two parallel max/max/mul chains over the packed bf16
pair via SRC_*_HI, written through WR0_LO/WR0_HI), doubling DVE throughput
to ~2 elem/lane/cycle — the single biggest win in this kernel.

The true diagonal d1*d2 (d1 = relu(2t1+th5_1), d2 = sigmoid(relu(2t2+th5_2)))
is patched per channel with ONE strided copy_predicated over the [128, 8, 128]
diagonal view of the channel's [128, 8*N] mega-tile (hand-built AP with free
pattern [[N+128, 8], [1, 128]]), amortizing per-op overhead 8x.

Everything on-device is bf16 (harness tolerance 2e-2 >> bf16 rounding); the
host converts back to f32. This halves HBM write traffic (the DMA roofline)
and halves matmul LDWEIGHTS time. Stores are deferred one channel so the
in-order sync/scalar sequencers never stall Vector on a not-yet-patched
tile; the last channel stores per-tile to overlap the drain.
"""

import sys
import types

import ml_dtypes
import numpy as np

B, C, N, E = 2, 64, 1024, 32
NCORES = 8
EPC = B * E // NCORES  # 8 channels per core
P = 128
NB = N // P  # 8 row blocks
H = N // 2  # matmul moving free-dim limit is 512

# Try to engage the DVE 2X_1PORT perf mode for the custom op (bf16 packed
# operands). The same uop program is written into the perf-mode table
# slots; correctness is checked end-to-end by the harness.
USE_2X = True
_CACHE = {}


def _ensure_hook_shim():
    """Make trace=True safe even when antenv.axon_hooks is absent."""
    try:
        import antenv.axon_hooks  # noqa: F401
    except ImportError:
        mod = types.ModuleType("antenv.axon_hooks")
        mod.get_axon_ntff_profile_hook = lambda: None
        mod.set_axon_ntff_profile_hook = lambda h: None
        sys.modules["antenv.axon_hooks"] = mod


def _build_2x_uop(base):
    """2X_1PORT program for out = max(in0,s0)*max(in1,s1).

    In 2X mode each port read delivers a packed pair of bf16 elements; the
    low element enters via SRC_0/SRC_1, the high one via SRC_0_HI/SRC_1_HI.
    Two copies of the max/max/mul chain are placed across the 8 ALU blocks;
    results are written packed via WR0_LO / WR0_HI. Per-partition scalar
    consts are shared by both chains (both elements are in the same row).
    """
    from concourse.dve_uop import (
        AluInp, AluOp, DelayInp, ENABLE, InpSel, OutPath, OutSel, UopConfig,
    )

    u = UopConfig()
    # lane j>=1 feeds delay chain j-1 at block 0
    u.enable_input(InpSel.SRC_0, 1)      # chain 0
    u.enable_input(InpSel.CONST_0, 2)    # chain 1
    u.enable_input(InpSel.SRC_1, 3)      # chain 2
    u.enable_input(InpSel.CONST_1, 4)    # chain 3
    u.enable_input(InpSel.SRC_0_HI, 5)   # chain 4
    u.enable_input(InpSel.SRC_1_HI, 6)   # chain 5
    u.require_inp0 = ENABLE
    u.require_inp1 = ENABLE
    u.trigger = base.trigger
    u.next_uop = (0, 0, 0)
    u.repeat_count = base.repeat_count

    dp = u.datapath_config
    # low chain
    dp[0].enable_alu(AluOp.MAX, AluInp.PREV_DELAY_0, AluInp.PREV_DELAY_1)
    dp[0].pass_through_delay(1, 2, 3, 4, 5)
    dp[1].enable_alu(AluOp.MAX, AluInp.PREV_DELAY_2, AluInp.PREV_DELAY_3)
    dp[1].enable_delay_from_src(DelayInp.PREV_ALU_OUT, 0)  # max_v_lo
    dp[1].pass_through_delay(1, 3, 4, 5)
    dp[2].enable_alu(AluOp.MULTIPLY, AluInp.PREV_DELAY_0, AluInp.PREV_ALU_OUT)
    dp[2].pass_through_delay(1, 3, 4, 5)
    # high chain (consts still on chains 1 and 3)
    dp[3].enable_alu(AluOp.MAX, AluInp.PREV_DELAY_4, AluInp.PREV_DELAY_1)
    dp[3].enable_delay_from_src(DelayInp.PREV_ALU_OUT, 0)  # out_lo
    dp[3].pass_through_delay(3, 5)
    dp[4].enable_alu(AluOp.MAX, AluInp.PREV_DELAY_5, AluInp.PREV_DELAY_3)
    dp[4].enable_delay_from_src(DelayInp.PREV_ALU_OUT, 1)  # max_v_hi
    dp[4].pass_through_delay(0)
    dp[5].enable_alu(AluOp.MULTIPLY, AluInp.PREV_DELAY_1, AluInp.PREV_ALU_OUT)
    dp[5].pass_through_delay(0)
    dp[6].pass_through_alu()
    dp[6].pass_through_delay(0)
    dp[7].pass_through_alu()
    dp[7].pass_through_delay(0)

    u.out[OutPath.WR0_LO] = OutSel.DELAY_0
    u.out_enable[OutPath.WR0_LO] = ENABLE
    u.out[OutPath.WR0_HI] = OutSel.ALU_OUT
    u.out_enable[OutPath.WR0_HI] = ENABLE
    return u


def _register_gated_maxmul():
    """Register the fused out = max(in0,s0)*max(in1,s1) custom DVE op."""
    import concourse.dve_ops as dve_ops
    from concourse.dve_ops import DveOp, OPS, has_src1, _COMPILE_CACHE
    from concourse.dve_spec import C0, C1, Spec, Src0, Src1, lower, maxx
    from concourse.dve_uop import DveOpSpec

    for op in OPS:
        if op.name == "GATED_MAXMUL_ANT":
            return op

    spec = Spec(
        body=maxx(Src0, C0) * maxx(Src1, C1),
        reference=lambda in0, in1, s0, s1, imm2: np.maximum(in0, s0)
        * np.maximum(in1, s1),
    )
    op = DveOp("GATED_MAXMUL_ANT", spec, subdim=False, uops_sha={})
    OPS.append(op)
    # Rebuild the registry views that were snapshotted at import time.
    dve_ops.CUSTOM_DVE_SPECS[op.name] = op.spec
    opcode = dve_ops._CUSTOM_DVE_ROW_BASE + len(OPS) - 1
    assert opcode < 0x20
    dve_ops._SUB_OPCODE_FOR_NAME[op.name] = opcode
    # Pre-seed the compile cache with a spec that (optionally) carries the
    # perf-mode uop programs; compile() then returns it without the sha check.
    for ver in ("v3", "v4"):
        uops = lower(spec, ver=ver)
        kw = {}
        if USE_2X:
            kw = dict(uops_2x=[_build_2x_uop(uops[0])])
        s = DveOpSpec(
            name=op.name, opcode=opcode, uops=uops,
            rd1_en=has_src1(spec), **kw,
        )
        op.uops_sha[ver] = s.sha(ver)
        _COMPILE_CACHE[(op.name, ver)] = s
    return op


def _build_program():
    import concourse.bacc as bacc
    import concourse.mybir as mybir
    import concourse.tile as tile
    from concourse.ap import AP

    f32 = mybir.dt.float32
    bf16 = mybir.dt.bfloat16
    AF = mybir.ActivationFunctionType
    ALU = mybir.AluOpType

    gated_op = _register_gated_maxmul()

    nc = bacc.Bacc("TRN2", target_bir_lowering=False, debug=False, num_devices=NCORES)

    emb = nc.declare_dram_parameter("emb", [C, N], bf16, isOutput=False)
    w = nc.declare_dram_parameter("w", [C, 40], bf16, isOutput=False)
    sel = nc.declare_dram_parameter("sel", [40, N], bf16, isOutput=False)
    th5bc = nc.declare_dram_parameter("th5bc", [P, NB * 2 * EPC], f32, isOutput=False)
    eye8 = nc.declare_dram_parameter("eye8", [P, NB * P], bf16, isOutput=False)
    out = nc.declare_dram_parameter("out", [EPC, N, N], bf16, isOutput=True)

    def custom(out_ap, in0, in1, s0, s1):
        bi = nc.vector._custom_dve(gated_op, out=out_ap, in0=in0, in1=in1, s0=s0, s1=s1)
        if USE_2X:
            bi.ins.perf_max = 1  # engine may escalate to 2X_1PORT
        return bi

    with tile.TileContext(nc, pool_alloc_mode="queue") as tc:
        with (
            tc.tile_pool(name="const", bufs=1) as cpool,
            tc.tile_pool(name="rows", bufs=1) as rpool,
        ):
            # Loads: trigger cost is ~0.6us each on the issuing sequencer
            # and transfers are line-count-bound, so spread the triggers
            # over sync+scalar and split the line-heavy tensors.
            sb_warm = cpool.tile([1, EPC], f32)
            nc.vector.memset(sb_warm[:], 0.0)
            # Warm the ACT sigmoid table first, during the input loads.
            nc.scalar.activation(sb_warm[:], sb_warm[:], AF.Sigmoid)
            sb_emb = cpool.tile([C, N], bf16)
            Q = N // 4
            for q in range(4):
                eng = nc.sync if q < 2 else nc.scalar
                eng.dma_start(out=sb_emb[:, q * Q:(q + 1) * Q],
                              in_=emb[:, q * Q:(q + 1) * Q])
            sb_w = cpool.tile([C, 40], bf16)
            nc.sync.dma_start(out=sb_w[0:32, :], in_=w[0:32, :])
            nc.sync.dma_start(out=sb_w[32:64, :], in_=w[32:64, :])
            sb_sel = cpool.tile([40, N], bf16)
            nc.scalar.dma_start(out=sb_sel[:, 0:H], in_=sel[:, 0:H])
            nc.scalar.dma_start(out=sb_sel[:, H:N], in_=sel[:, H:N])
            sb_th5bc = cpool.tile([P, NB, 2 * EPC], f32)
            nc.scalar.dma_start(out=sb_th5bc[0:64, :, :], in_=th5bc[0:64, :])
            nc.scalar.dma_start(out=sb_th5bc[64:128, :, :], in_=th5bc[64:128, :])
            sb_eye8 = cpool.tile([P, NB, P], bf16)
            nc.scalar.dma_start(out=sb_eye8[0:64, :, :], in_=eye8[0:64, :])
            nc.scalar.dma_start(out=sb_eye8[64:128, :, :], in_=eye8[64:128, :])

            # Row-space: vg rows (v on partitions 0-7, g on 32-39), bf16.
            # g sits at partition 32 because engine APs must start on a
            # quad (32-partition) boundary.
            sb_vg = rpool.tile([40, N], bf16)
            # Column-space: vgcol[:, r, 0:8] = v at node r*128+p, [:, r, 8:16] = g.
            sb_vgc = rpool.tile([P, NB, 2 * EPC], f32)
            sb_pdb = rpool.tile([P, NB, EPC], bf16)  # true diag d1*d2
            sb_u = rpool.tile([P, NB, 2 * EPC], f32)
            sb_d = rpool.tile([P, NB, 2 * EPC], f32)

            with (
                tc.tile_pool(name="psum", bufs=2, space="PSUM") as pp,
                tc.tile_pool(name="ph1ps", bufs=1, space="PSUM") as pt,
                tc.tile_pool(name="colps", bufs=1, space="PSUM") as cp,
                tc.tile_pool(name="jrepsb", bufs=2) as jsb,
                tc.tile_pool(name="work", bufs=4) as wp,
            ):
                # Rows: t = w.T @ emb -> [40, N] (t1 on 0-7, t2 on 32-39).
                ps_t = pt.tile([40, N], f32, tag="ps_t")
                for h in range(2):
                    hs = slice(h * H, (h + 1) * H)
                    nc.tensor.matmul(
                        ps_t[:, hs], lhsT=sb_w[:], rhs=sb_emb[:, hs],
                        start=True, stop=True,
                    )
                for h in range(2):
                    hs = slice(h * H, (h + 1) * H)
                    # relus on Vector (idle here) so Scalar's serial stream
                    # only carries the sigmoid in the head's critical path.
                    nc.vector.tensor_scalar(
                        sb_vg[0:EPC, hs], ps_t[0:EPC, hs], 2.0, 0.0,
                        op0=ALU.mult, op1=ALU.max,
                    )
                    nc.vector.tensor_scalar(
                        sb_vg[32:40, hs], ps_t[32:40, hs], 2.0, 0.0,
                        op0=ALU.mult, op1=ALU.max,
                    )
                    nc.scalar.activation(
                        sb_vg[32:40, hs], sb_vg[32:40, hs], AF.Sigmoid,
                    )

                # Channel 0's replication goes right behind phase 1 in the
                # PE stream so its vj/gj are ready with the first columns.
                sb_vj = jsb.tile([P, N], bf16, tag="sb_vj")
                sb_gj = jsb.tile([P, N], bf16, tag="sb_gj")
                for h in range(2):
                    hs = slice(h * H, (h + 1) * H)
                    ps_vh = pp.tile([P, H], f32, tag="ps_vh")
                    nc.tensor.matmul(
                        ps_vh[:], lhsT=sb_sel[0:EPC, 0:P],
                        rhs=sb_vg[0:EPC, hs], start=True, stop=True,
                    )
                    nc.scalar.copy(sb_vj[:, hs], ps_vh[:])
                    ps_gh = pp.tile([P, H], f32, tag="ps_gh")
                    nc.tensor.matmul(
                        ps_gh[:], lhsT=sb_sel[32:40, 0:P],
                        rhs=sb_vg[32:40, hs], start=True, stop=True,
                    )
                    nc.scalar.copy(sb_gj[:, hs], ps_gh[:])
                repl0 = (sb_vj, sb_gj)

                # Columns: tcol[p, r, k] = t_k[r*128+p] via emb-block matmuls.
                ps_c = cp.tile([P, NB, 40], f32, tag="ps_c")
                for r in range(NB):
                    nc.tensor.matmul(
                        ps_c[:, r, :], lhsT=sb_emb[:, r * P:(r + 1) * P],
                        rhs=sb_w[:], start=True, stop=True,
                    )
                # vgc = relu(2*tcol) (+ sigmoid on g half); u = 2*tcol + th5.
                # In r-halves so the first GATED isn't gated on column r7.
                for a, b in ((0, NB // 2), (NB // 2, NB)):
                    nc.vector.tensor_scalar(
                        sb_vgc[:, a:b, 0:EPC], ps_c[:, a:b, 0:EPC], 2.0, 0.0,
                        op0=ALU.mult, op1=ALU.max,
                    )
                    nc.vector.tensor_scalar(
                        sb_vgc[:, a:b, EPC:], ps_c[:, a:b, 32:40], 2.0, 0.0,
                        op0=ALU.mult, op1=ALU.max,
                    )
                    nc.scalar.activation(
                        sb_vgc[:, a:b, EPC:], sb_vgc[:, a:b, EPC:], AF.Sigmoid,
                    )
                    nc.vector.scalar_tensor_tensor(
                        sb_u[:, a:b, 0:EPC], ps_c[:, a:b, 0:EPC], 2.0,
                        sb_th5bc[:, a:b, 0:EPC], op0=ALU.mult, op1=ALU.add,
                    )
                    nc.vector.scalar_tensor_tensor(
                        sb_u[:, a:b, EPC:], ps_c[:, a:b, 32:40], 2.0,
                        sb_th5bc[:, a:b, EPC:], op0=ALU.mult, op1=ALU.add,
                    )

                pending = None  # (ch, mega-tile) stores deferred one iter
                for ch in range(EPC):
                    if ch == 0:
                        sb_vj, sb_gj = repl0
                    else:
                        # Replicate channel ch's v/g rows across partitions
                        # (indicator-matmul, K=8, bf16, 1 cyc/row).
                        sb_vj = jsb.tile([P, N], bf16, tag="sb_vj")
                        sb_gj = jsb.tile([P, N], bf16, tag="sb_gj")
                        for h in range(2):
                            hs = slice(h * H, (h + 1) * H)
                            ps_vh = pp.tile([P, H], f32, tag="ps_vh")
                            nc.tensor.matmul(
                                ps_vh[:],
                                lhsT=sb_sel[0:EPC, ch * P:(ch + 1) * P],
                                rhs=sb_vg[0:EPC, hs],
                                start=True, stop=True,
                            )
                            nc.scalar.copy(sb_vj[:, hs], ps_vh[:])
                            ps_gh = pp.tile([P, H], f32, tag="ps_gh")
                            nc.tensor.matmul(
                                ps_gh[:],
                                lhsT=sb_sel[32:40, ch * P:(ch + 1) * P],
                                rhs=sb_vg[32:40, hs],
                                start=True, stop=True,
                            )
                            nc.scalar.copy(sb_gj[:, hs], ps_gh[:])

                    # Flush the previous channel's stores now: its diag patch
                    # is already done, so the sequencers won't stall on it,
                    # and the vj/gj copies above stay ahead of Vector.
                    if pending is not None:
                        pch, po = pending
                        for r in range(NB):
                            eng = nc.sync if r % 2 == 0 else nc.scalar
                            eng.dma_start(
                                out=out[pch, r * P:(r + 1) * P, :],
                                in_=po[:, r, :],
                            )
                        pending = None

                    # One mega-tile per channel; DVE fills the 8 row blocks.
                    last = ch == EPC - 1
                    o = wp.tile([P, NB, N], bf16, tag="o")
                    for r in range(NB):
                        custom(
                            o[:, r, :], sb_vj[:], sb_gj[:],
                            sb_vgc[:, r, ch:ch + 1],
                            sb_vgc[:, r, EPC + ch:EPC + ch + 1],
                        )
                        if last:
                            # Per-tile patch + store so the drain overlaps
                            # the remaining compute instead of trailing it.
                            nc.vector.copy_predicated(
                                o[:, r, r * P:(r + 1) * P],
                                sb_eye8[:, r, :].bitcast(mybir.dt.int16),
                                sb_pdb[:, r, ch:ch + 1].broadcast_to([P, P]),
                            )
                            nc.sync.dma_start(
                                out=out[ch, r * P:r * P + 64, :],
                                in_=o[0:64, r, :],
                            )
                            nc.scalar.dma_start(
                                out=out[ch, r * P + 64:(r + 1) * P, :],
                                in_=o[64:128, r, :],
                            )
                    if ch == 0:
                        # Deferred diag-value chain: relu on Vector, only the
                        # sigmoid needs Scalar, so neither engine stalls.
                        nc.vector.tensor_scalar_max(sb_d[:], sb_u[:], 0.0)
                        nc.scalar.activation(sb_d[:, :, EPC:], sb_d[:, :, EPC:],
                                             AF.Sigmoid)
                        nc.vector.tensor_mul(sb_pdb[:], sb_d[:, :, :EPC],
                                             sb_d[:, :, EPC:])

                    if not last:
                        # Batched diagonal patch: one strided op per channel.
                        # Diag of row block r lives at free offset r*N + r*P,
                        # i.e. the [[N+P, NB], [1, P]] strided view.
                        full = o[:]
                        pairs = [list(p) for p in full.ap]
                        dview = AP(full.tensor, full.offset,
                                   [pairs[0], [N + P, NB], [1, P]])
                        nc.vector.copy_predicated(
                            dview, sb_eye8[:].bitcast(mybir.dt.int16),
                            sb_pdb[:, :, ch:ch + 1].broadcast_to([P, NB, P]),
                        )
                        pending = (ch, o)

                # Flush any channel whose stores are still deferred.
                if pending is not None:
                    pch, po = pending
                    for r in range(NB):
                        eng = nc.sync if r % 2 == 0 else nc.scalar
                        eng.dma_start(out=out[pch, r * P:(r + 1) * P, :],
                                      in_=po[:, r, :])

    nc.compile()
    return nc


def _get_program():
    if "nc" not in _CACHE:
        _CACHE["nc"] = _build_program()
    return _CACHE["nc"]


def kernel(**inputs):
    _ensure_hook_shim()
    from concourse.bass_utils import run_bass_kernel_spmd

    bf = ml_dtypes.bfloat16
    emb = np.ascontiguousarray(np.asarray(inputs["emb"], dtype=np.float32)).astype(bf)
    th12_1 = np.asarray(inputs["th12_1"], dtype=np.float32)
    th12_2 = np.asarray(inputs["th12_2"], dtype=np.float32)
    th5_1 = np.asarray(inputs["th5_1"], dtype=np.float32)
    th5_2 = np.asarray(inputs["th5_2"], dtype=np.float32)
    eye8 = np.tile(np.eye(P, dtype=np.float32), (1, NB)).astype(bf)

    # sel[k, ch*128+m] = (k==ch) for k<8 and (k-32==ch) for 32<=k<40
    sel = np.zeros((40, N), dtype=bf)
    for ch in range(EPC):
        sel[ch, ch * P:(ch + 1) * P] = 1
        sel[32 + ch, ch * P:(ch + 1) * P] = 1

    in_maps = []
    for k in range(NCORES):
        b = k // (NCORES // B)
        e0 = (k % (NCORES // B)) * EPC
        w = np.zeros((C, 40), dtype=bf)
        w[:, 0:EPC] = th12_1[e0:e0 + EPC].T.astype(bf)
        w[:, 32:40] = th12_2[e0:e0 + EPC].T.astype(bf)
        th5cat = np.concatenate([th5_1[e0:e0 + EPC], th5_2[e0:e0 + EPC]])  # [16]
        th5bc = np.tile(th5cat[None, :], (P, NB)).astype(np.float32)  # [128, 128]
        in_maps.append(
            {
                "emb": np.ascontiguousarray(emb[b]),
                "w": np.ascontiguousarray(w),
                "sel": sel,
                "th5bc": th5bc,
                "eye8": eye8,
            }
        )

    nc = _get_program()
    res = run_bass_kernel_spmd(nc, in_maps, core_ids=list(range(NCORES)))
    _CACHE["last_result"] = res

    out = np.empty((B, E, N, N), dtype=np.float32)
    for k in range(NCORES):
        b = k // (NCORES // B)
        e0 = (k % (NCORES // B)) * EPC
        out[b, e0:e0 + EPC] = np.asarray(res.results[k]["out"], dtype=np.float32)
    return out
